# revision 37
# baseline (speedup 1.0000x reference)
"""GQA attention (llama-style, RoPE, causal) on 8 Trainium2 NeuronCores.

Problem: B=2, S=2048, DIM=2048, 16 q-heads / 4 kv-heads, head_dim=128.

Sharding: batch x kv-group. Core c handles batch b=c//4 and kv-group
g=c%4 (q-heads 4g..4g+3, kv-head g). Each core computes its 4 heads'
attention and a partial output projection against wo[:, 512g:512(g+1)];
the host sums the 4 partials per batch. No cross-core communication.

Device-side layout is fully "transposed": activations live as [dim, seq]
so every matmul's contraction dim sits on the SBUF partition axis.
All matmuls run in bf16 (1 cyc/row on PE, same as fp32r, but half the
DMA bytes and 2-4x DVE throughput on the element-wise work).

v2 changes vs the fp32r baseline (339us measured):
 - The per-k-block softmax-sum matmuls (ones^T @ exp, 160 of them, ~36us
   of PE) are gone. Exp tiles accumulate element-wise on the Vector
   engine into two bf16 accumulators (even/odd k-blocks, bounded
   rounding depth); one ones-matmul per head (16 total) does the final
   128-partition reduction. 1/sums via DVE reciprocal_approx_fast.
 - With the sums removed, a head's attention inner loop is 454ns/blk of
   PE vs 612ns/blk of ACT (exp) — so attention alone would be
   ACT-paced. The emission therefore software-pipelines: Q-projection
   of head h+1, K/V projections of chunk c+1 and the output projection
   of chunk c-1 are split into per-matmul "fillers" interleaved between
   attention blocks, keeping PE the pacing engine everywhere.
 - y partials stored as bf16 (half the store drain).
"""

import numpy as np
from contextlib import ExitStack

import bass_rust
import concourse.bass as bass
import concourse.mybir as mybir
import concourse.tile as tile
from concourse.bass_utils import run_bass_kernel_spmd

P = 128          # SBUF partitions / head_dim
S = 2048         # sequence length
D = 2048         # model dim
KC = 16          # contraction chunks of 128 over D
SC = 4           # s-chunks of 512
QW = 512         # moving-operand width
NH = 4           # q-heads per core
N_CORES = 8
SCALE = float(1.0 / np.sqrt(np.float32(128.0)))
F32 = mybir.dt.float32
F32R = mybir.dt.float32r
BF16 = mybir.dt.bfloat16
EXP = mybir.ActivationFunctionType.Exp
LN = mybir.ActivationFunctionType.Ln


class _TC(tile.TileContext):
    """TileContext whose tail drain splits its semaphore waits into
    separate wait instructions — the walrus build here rejects a Drain
    carrying more than a couple of inline sync waits."""

    def _drain_and_barrier(self, tick_clock, wait_clock):
        gc = tick_clock.global_clock
        ticks = [gc[i] for i in range(27)]
        for proc, sem in sorted(self.sems.allocated().items()):
            t = ticks[proc]
            if t > 0:
                mult = 16 if sem.name.startswith(("DMAHW", "DMASW")) else 1
                self.nc.sync.wait_ge(sem, t * mult)
        self.nc.sync.drain()
        self.nc.all_engine_barrier()
        popped = self.nc._tile_sem_poison_stack.pop()
        assert popped is self._sem_poison
        self.nc.clear_and_free_semaphores(list(self.sems.allocated().values()))
        self.nc.all_engine_barrier()


def _split_excess_waits(nc, max_waits=1):
    """This walrus build allows very few inline sync waits per TPB
    instruction. Move excess waits onto injected same-engine NOPs placed
    just before the instruction — semantically identical, since the
    engine queue executes in order."""
    for f in nc.m.functions:
        for blk in f.blocks:
            insts = blk.instructions
            new_list = []
            for inst in insts:
                si = inst.sync_info
                if si is not None and len(si.on_wait) > max_waits:
                    waits = list(si.on_wait)
                    excess, keep = waits[:-max_waits], waits[-max_waits:]
                    for j, w in enumerate(excess):
                        nop = bass_rust.InstNoOp(name=f"{inst.name}-wn{j}")
                        nop.engine = inst.engine
                        nop.sync_info = bass_rust.SyncInfo(
                            on_wait=[w], on_update=[])
                        new_list.append(nop)
                    inst.sync_info = bass_rust.SyncInfo(
                        on_wait=keep, on_update=list(si.on_update))
                new_list.append(inst)
            insts[:] = new_list


def _emit(nc, tc, ctx, t):
    pool = lambda name, bufs, space="SBUF": ctx.enter_context(
        tc.tile_pool(name=name, bufs=bufs, space=space)
    )

    # SBUF pools
    xp = pool("xp", 9)          # x chunk groups [128, 2048] bf16, 2 chunks live
    constp = pool("constp", 1)  # weights, trig tables, masks, resident slabs
    qsbp = pool("qsbp", 2)      # pre-rope proj copy
    t1p = pool("t1p", 2)
    t2p = pool("t2p", 2)
    qrp = pool("qrp", 3)        # rope'd q tiles
    vsbp = pool("vsbp", 1)      # pre-transpose v copy
    ep = pool("ep", 4)          # exp tiles
    eaccp = pool("eaccp", 4)    # exp accumulators (2 per head, 2 heads live)
    rp = pool("rp", 2)          # reciprocal [1, 512]
    rbp = pool("rbp", 2)        # broadcast recip [128, 512]
    otp = pool("otp", 9)        # normalized attention out, 2 chunks live
    yp = pool("yp", 3)          # output copy slabs [128, 1024] bf16

    # PSUM pools — 8 banks total
    pacc = pool("pacc", 2, "PSUM")    # proj accumulators        (2 banks)
    pscore = pool("pscore", 2, "PSUM")  # scoresT                (2 banks)
    pout = pool("pout", 2, "PSUM")    # attention out accum      (2 banks)
    yps = pool("yps", 1, "PSUM")      # output proj y tiles      (1 bank)
    shp = pool("shp", 1, "PSUM")      # rope swap / v transp / bcast / psm (1)

    # resident SBUF slabs
    wq_sb = constp.tile([P, KC * 4 * P], BF16, tag="wq")  # chunk (h,k) at h*2048+k*128
    wk_sb = constp.tile([P, KC * P], BF16, tag="wk")      # chunk k at k*128
    wv_sb = constp.tile([P, KC * P], BF16, tag="wv")
    wo_sb = constp.tile([P, NH * S], BF16, tag="wo")      # chunk (h,dm) at h*2048+dm*128
    cm_sb = constp.tile([P, 4 * QW], BF16, tag="cm")      # 4 causal masks
    cos_sb = constp.tile([P, S], BF16, tag="cos")
    sin_sb = constp.tile([P, S], BF16, tag="sin")
    perm_sb = constp.tile([P, P], BF16, tag="perm")       # pair-swap permutation
    ident_sb = constp.tile([P, P], BF16, tag="ident")
    ones_sb = constp.tile([P, 1], BF16, tag="ones")
    onesrow_sb = constp.tile([1, P], F32R, tag="onesrow")
    wup_sb = constp.tile([P, QW], BF16, tag="wup")        # warmup junk

    kT_sb = constp.tile([P, S], BF16, tag="kT")    # rope'd K^T, filled per s-chunk
    vnat_sb = constp.tile([P, S], BF16, tag="vn")  # V natural [kpos, d], 16 col-blocks

    xT_d, yT_d = t["xT"], t["yT"]

    xgs = {}       # sc -> list of 4 xg tiles (each [128, 4*512])
    qr_tiles = {}  # h -> rope'd q tile for the current chunk
    o_tiles = {}   # (sc, h) -> normalized attention out tile
    norm_ref = {}  # (sc, h) -> (po psum, r recip tile)

    def load_xgroups(sc, split=False):
        # x rides the Activation engine's HW DGE queue — a second hardware
        # DMA queue in parallel with the sync-engine queue the weights use
        tiles = []
        for g in range(4):
            xg = xp.tile([P, 4 * QW], BF16, tag="xg")
            ssl = slice(QW * sc, QW * (sc + 1))
            if split:
                for j in range(4):
                    nc.scalar.dma_start(
                        xg[:, QW * j:QW * (j + 1)],
                        xT_d[P * (4 * g + j):P * (4 * g + j + 1), ssl])
            else:
                src = xT_d[4 * P * g:4 * P * (g + 1), ssl].rearrange(
                    "(k p) s -> p k s", p=P)
                nc.scalar.dma_start(xg[:].rearrange("p (k s) -> p k s", k=4), src)
            tiles.append(xg)
        xgs[sc] = tiles

    def xs_of(sc):
        g = xgs[sc]
        return [g[k // 4][:, QW * (k % 4):QW * (k % 4 + 1)] for k in range(KC)]

    # ---- filler generators: lists of zero-arg closures, one PE matmul each
    def mk_proj(sc, pi):
        """pi 0..3 = Q head pi (chunk sc); 4 = K (chunk sc); 5 = V (chunk sc).
        Returns 16 mm closures; the 17th emits the post-group chain
        (psum eviction + rope or v-transpose)."""
        st = {}
        xs = None

        def w_ap(k):
            if pi < 4:
                base = pi * 2048 + k * P
                return wq_sb[:, base:base + P]
            if pi == 4:
                return wk_sb[:, k * P:(k + 1) * P]
            return wv_sb[:, k * P:(k + 1) * P]

        def mm(k):
            nonlocal xs
            if k == 0:
                st["ps"] = pacc.tile([P, QW], F32, tag="acc", name="acc")
                xs = xs_of(sc)
            nc.tensor.matmul(st["ps"][:], w_ap(k), xs[k],
                             start=(k == 0), stop=(k == KC - 1))

        def chain():
            ps = st["ps"]
            ssl = slice(QW * sc, QW * (sc + 1))
            if pi == 5:
                # V: psum -> sbuf, then PE-transpose 128-blocks into vnat
                vsb = vsbp.tile([P, QW], BF16, tag="vsb")
                nc.scalar.copy(vsb[:], ps[:])
                for tb in range(4):
                    pt = shp.tile([P, P], BF16, tag="sh")
                    nc.tensor.transpose(pt[:], vsb[:, P * tb:P * (tb + 1)],
                                        ident_sb[:])
                    blk = 4 * sc + tb
                    nc.scalar.copy(vnat_sb[:, P * blk:P * (blk + 1)], pt[:])
            else:
                # Q/K: rope = psum*cos2 + (perm @ psum)*sin2
                qsb = qsbp.tile([P, QW], BF16, tag="qsb")
                nc.scalar.copy(qsb[:], ps[:])
                sw = shp.tile([P, QW], F32, tag="sh")
                nc.tensor.matmul(sw[:], perm_sb[:], qsb[:],
                                 start=True, stop=True)
                t1 = t1p.tile([P, QW], BF16, tag="t1")
                nc.vector.tensor_mul(t1[:], qsb[:], cos_sb[:, ssl])
                t2 = t2p.tile([P, QW], BF16, tag="t2")
                nc.vector.tensor_mul(t2[:], sw[:], sin_sb[:, ssl])
                if pi < 4:
                    dst = qrp.tile([P, QW], BF16, tag="qr")
                    qr_tiles[pi] = dst
                    nc.vector.tensor_add(dst[:], t1[:], t2[:])
                else:
                    nc.vector.tensor_add(kT_sb[:, ssl], t1[:], t2[:])

        return [lambda k=k: mm(k) for k in range(KC)] + [chain]

    def mk_oproj(sc, alt_pool=None):
        """Output projection for chunk sc (64 mm closures; evictions and
        stores ride along on the closing matmul of each 4-mm group).
        alt_pool: alternate even dm groups into another (idle) PSUM pool
        so back-to-back groups don't serialize on the single y bank."""
        st = {}
        ssl = slice(QW * sc, QW * (sc + 1))

        def mm(dm, h):
            if h == 0:
                if alt_pool is not None and dm % 2 == 0:
                    st["py"] = alt_pool.tile([P, QW], F32, tag="score",
                                             name="py")
                else:
                    st["py"] = yps.tile([P, QW], F32, tag="y", name="py")
            nc.tensor.matmul(
                st["py"][:], wo_sb[:, S * h + P * dm:S * h + P * (dm + 1)],
                o_tiles[(sc, h)][:], start=(h == 0), stop=(h == NH - 1))
            if h == NH - 1:
                if dm % 2 == 0:
                    st["ysb"] = yp.tile([P, 2 * QW], BF16, tag="ysb", name="ysb")
                nc.vector.tensor_copy(
                    st["ysb"][:, QW * (dm % 2):QW * (dm % 2 + 1)], st["py"][:])
                if dm % 2 == 1:
                    for j in range(2):
                        dmj = dm - 1 + j
                        nc.sync.dma_start(
                            yT_d[P * dmj:P * (dmj + 1), ssl],
                            st["ysb"][:, QW * j:QW * (j + 1)])

        return [lambda dm=dm, h=h: mm(dm, h) for dm in range(KC) for h in range(NH)]

    def emit_norm(key):
        # broadcast 1/sums across partitions via a K=1 matmul; by the
        # time this runs on PE, r has long been ready (no PE stall)
        po, r = norm_ref.pop(key)
        rbp_ps = shp.tile([P, QW], F32, tag="sh")
        nc.tensor.matmul(rbp_ps[:], onesrow_sb[:], r[:], start=True, stop=True)
        rb = rbp.tile([P, QW], F32, tag="rb")
        nc.vector.tensor_copy(rb[:], rbp_ps[:])
        ot = otp.tile([P, QW], BF16, tag="ot")
        o_tiles[key] = ot
        nc.vector.tensor_mul(ot[:], po[:], rb[:])

    def emit_slot(sc, h, fillers, norm_key):
        """Attention for head h of chunk sc, with fillers interleaved."""
        nkb = 4 * sc + 4
        qr = qr_tiles[h]
        po = pout.tile([P, QW], F32, tag="out")
        eA = eaccp.tile([P, QW], BF16, tag="eacc")
        eB = eaccp.tile([P, QW], BF16, tag="eacc")
        prev = None
        fi = 0

        def pop(n):
            nonlocal fi
            for _ in range(n):
                if fi < len(fillers):
                    fillers[fi]()
                    fi += 1

        def pop_paced(kb):
            # spread remaining fillers over remaining blocks (cap 4/blk),
            # reserving 4 to cover the slot-end exp-sum merge latency
            rem_kb = nkb - kb
            n = min(4, max(0, -(-(len(fillers) - fi - 4) // rem_kb)))
            pop(n)

        # q-columns < 128*kb - 512*sc are causally dead for k-block kb, so
        # the last three k-blocks shrink their moving width (384/256/128)
        qlo = lambda kb: max(0, P * kb - QW * sc)
        for kb in range(nkb):
            ql = qlo(kb)
            qsl = slice(ql, QW)
            psc = pscore.tile([P, QW], F32, tag="score")
            nc.tensor.matmul(psc[:, qsl], kT_sb[:, P * kb:P * (kb + 1)],
                             qr[:, qsl], start=True, stop=True)
            tgt = eA if kb == 0 else eB if kb == 1 else ep.tile(
                [P, QW], BF16, tag="exp")
            nc.scalar.activation(tgt[:, qsl], psc[:, qsl], EXP, scale=SCALE)
            off = P * kb - QW * sc
            if off >= 0:  # diagonal-overlap block: zero kpos > q
                j = off // P
                nc.vector.tensor_mul(tgt[:, qsl], tgt[:, qsl],
                                     cm_sb[:, QW * j + ql:QW * (j + 1)])
            if kb >= 2:
                acc = eA if kb % 2 == 0 else eB
                nc.vector.tensor_add(acc[:, qsl], acc[:, qsl], tgt[:, qsl])
            if kb == min(3, nkb - 1) and norm_key is not None:
                emit_norm(norm_key)
            pop_paced(kb)
            if prev is not None:
                pkb, ptgt = prev
                psl = slice(qlo(pkb), QW)
                nc.tensor.matmul(po[:, psl], vnat_sb[:, P * pkb:P * (pkb + 1)],
                                 ptgt[:, psl], start=(pkb == 0), stop=False)
            prev = (kb, tgt)
        pkb, ptgt = prev
        psl = slice(qlo(pkb), QW)
        nc.tensor.matmul(po[:, psl], vnat_sb[:, P * pkb:P * (pkb + 1)],
                         ptgt[:, psl], start=(pkb == 0), stop=True)
        # final exp-sum: eA += eB on DVE (over eB's covered columns), then
        # one 128->1 reduction matmul. Fillers sit between the DVE merge
        # and the reduction so PE never waits on the merge latency.
        q1 = qlo(1)
        nc.vector.tensor_add(eA[:, q1:], eA[:, q1:], eB[:, q1:])
        pop(4)
        psm = shp.tile([1, QW], F32, tag="sh")
        nc.tensor.matmul(psm[:], ones_sb[:], eA[:], start=True, stop=True)
        # 1/sums = exp(-ln(sums)) on ScalarE (ACT-only: PE keeps going)
        lnr = rp.tile([1, QW], F32, tag="r")
        nc.scalar.activation(lnr[:], psm[:], LN)
        r = rp.tile([1, QW], F32R, tag="r")
        nc.scalar.activation(r[:], lnr[:], EXP, scale=-1.0)
        norm_ref[(sc, h)] = (po, r)
        pop(len(fillers))  # flush

    # ---------------- kernel body ----------------
    # chunk 0 const/weight DMAs; x for chunk 0 split per 128-col chunk so
    # the first K-proj matmuls can start as soon as possible.
    nc.sync.dma_start(perm_sb[:], t["perm"][:])
    nc.sync.dma_start(ident_sb[:], t["ident"][:])
    nc.sync.dma_start(wup_sb[:], t["wup"][:])
    nc.sync.dma_start(ones_sb[:], t["onescol"][:])
    nc.sync.dma_start(onesrow_sb[:], t["onesrow"][:])
    # wk quarters on the sync queue; x pieces in parallel on the ACT queue
    xg0 = []
    for g in range(4):
        wqt = slice(4 * P * g, 4 * P * (g + 1))
        nc.sync.dma_start(wk_sb[:, wqt], t["wk"][:, wqt])
        xg = xp.tile([P, 4 * QW], BF16, tag="xg", name="xg")
        for j in range(4):
            nc.scalar.dma_start(
                xg[:, QW * j:QW * (j + 1)],
                xT_d[P * (4 * g + j):P * (4 * g + j + 1), 0:QW])
        xg0.append(xg)
    xgs[0] = xg0
    for g in range(4):
        wqt = slice(4 * P * g, 4 * P * (g + 1))
        nc.sync.dma_start(wv_sb[:, wqt], t["wv"][:, wqt])
    nc.sync.dma_start(wq_sb[:, 0:2048], t["wq"][:, 0:2048])
    for h in range(1, NH):
        nc.sync.dma_start(wq_sb[:, 2048 * h:2048 * (h + 1)],
                          t["wq"][:, 2048 * h:2048 * (h + 1)])
    nc.sync.dma_start(cos_sb[:], t["cos2"][:])
    nc.sync.dma_start(sin_sb[:], t["sin2"][:])
    nc.sync.dma_start(cm_sb[:], t["cm"][:])
    for h in range(NH):
        nc.sync.dma_start(wo_sb[:, S * h:S * (h + 1)],
                          t["woT"][P * h:P * (h + 1), :])

    # warmup burst: junk matmuls heat the PE clock while DMA ramps
    wupp = shp.tile([P, QW], F32, tag="sh")
    for _ in range(40):
        nc.tensor.matmul(wupp[:], perm_sb[:], wup_sb[:], start=True, stop=True)

    # chunk 0 preamble: K(0), V(0), Q(0,0) contiguous
    for fl in mk_proj(0, 4) + mk_proj(0, 5) + mk_proj(0, 0):
        fl()

    for sc in range(SC):
        if sc < SC - 1:
            load_xgroups(sc + 1)  # prefetch next chunk's x
        # per-slot filler lists. Q(sc,h+1) leads slot h (qr needed next
        # slot); K(sc+1)/Q(sc+1,0) go in slot 2 so their rope chains land
        # well before chunk sc+1's first scores; oproj(sc-1) fills the
        # gaps (its first group is ordered after Q so it pops after the
        # (sc-1,3) normalization has been emitted at kb==3).
        slot_fill = [[], [], [], []]
        slot_fill[0] += mk_proj(sc, 1)
        slot_fill[1] += mk_proj(sc, 2)
        slot_fill[2] += mk_proj(sc, 3)
        if sc < SC - 1:
            slot_fill[2] += mk_proj(sc + 1, 4) + mk_proj(sc + 1, 0)
            slot_fill[3] += mk_proj(sc + 1, 5)
        if sc > 0:
            op = mk_oproj(sc - 1)
            slot_fill[0] += op[:12]
            slot_fill[1] += op[12:28]
            slot_fill[2] += op[28:44]
            slot_fill[3] += op[44:]
        for h in range(NH):
            if h > 0:
                norm_key = (sc, h - 1)
            elif sc > 0:
                norm_key = (sc - 1, 3)
            else:
                norm_key = None
            emit_slot(sc, h, slot_fill[h], norm_key)

    # tail: last head's normalization + output projection of chunk 3.
    # pacc/pscore are idle here — run a 4-deep software pipeline across
    # four banks: each group's bank has a full 4-matmul group of slack
    # before reuse, so the eviction latency never blocks PE. Holding each
    # group's closing (h=3) matmul back also hides the (3,3)
    # normalization latency.
    emit_norm((SC - 1, 3))
    ssl3 = slice(QW * (SC - 1), QW * SC)
    pools3 = [pscore, pscore, yps, pacc]
    open_groups = []
    ysb_st = {}

    def close_group(dm, py):
        nc.tensor.matmul(
            py[:], wo_sb[:, S * 3 + P * dm:S * 3 + P * (dm + 1)],
            o_tiles[(SC - 1, 3)][:], start=False, stop=True)
        if dm % 2 == 0:
            ysb_st["ysb"] = yp.tile([P, 2 * QW], BF16, tag="ysb", name="ysb")
        # alternate eviction engine so consecutive groups' copies overlap
        if dm % 2 == 0:
            nc.vector.tensor_copy(
                ysb_st["ysb"][:, QW * (dm % 2):QW * (dm % 2 + 1)], py[:])
        else:
            nc.scalar.copy(
                ysb_st["ysb"][:, QW * (dm % 2):QW * (dm % 2 + 1)], py[:])
        if dm % 2 == 1:
            for j in range(2):
                dmj = dm - 1 + j
                nc.sync.dma_start(yT_d[P * dmj:P * (dmj + 1), ssl3],
                                  ysb_st["ysb"][:, QW * j:QW * (j + 1)])

    tags3 = {id(pscore): "score", id(yps): "y", id(pacc): "acc"}
    for dm in range(KC):
        pl = pools3[dm % 4]
        py = pl.tile([P, QW], F32, tag=tags3[id(pl)], name="py")
        for h in range(3):
            nc.tensor.matmul(
                py[:], wo_sb[:, S * h + P * dm:S * h + P * (dm + 1)],
                o_tiles[(SC - 1, h)][:], start=(h == 0), stop=False)
        open_groups.append((dm, py))
        if len(open_groups) == 4:
            close_group(*open_groups.pop(0))
    while open_groups:
        close_group(*open_groups.pop(0))


def build():
    nc = bass.Bass("TRN2", target_bir_lowering=False, debug=False,
                   num_devices=N_CORES)
    t = {
        "xT": nc.dram_tensor("xT", [D, S], BF16, kind="ExternalInput"),
        "wq": nc.dram_tensor("wq", [P, KC * 4 * P], BF16, kind="ExternalInput"),
        "wk": nc.dram_tensor("wk", [P, KC * P], BF16, kind="ExternalInput"),
        "wv": nc.dram_tensor("wv", [P, KC * P], BF16, kind="ExternalInput"),
        "woT": nc.dram_tensor("woT", [NH * P, S], BF16, kind="ExternalInput"),
        "cos2": nc.dram_tensor("cos2", [P, S], BF16, kind="ExternalInput"),
        "sin2": nc.dram_tensor("sin2", [P, S], BF16, kind="ExternalInput"),
        "cm": nc.dram_tensor("cm", [P, 4 * QW], BF16, kind="ExternalInput"),
        "perm": nc.dram_tensor("perm", [P, P], BF16, kind="ExternalInput"),
        "ident": nc.dram_tensor("ident", [P, P], BF16, kind="ExternalInput"),
        "onescol": nc.dram_tensor("onescol", [P, 1], BF16, kind="ExternalInput"),
        "onesrow": nc.dram_tensor("onesrow", [1, P], F32R, kind="ExternalInput"),
        "wup": nc.dram_tensor("wup", [P, QW], BF16, kind="ExternalInput"),
        "yT": nc.dram_tensor("yT", [D, S], BF16, kind="ExternalOutput"),
    }
    aps = {k: v.ap() for k, v in t.items()}
    with _TC(nc, num_cores=N_CORES) as tc:
        with ExitStack() as ctx:
            _emit(nc, tc, ctx, aps)
    _split_excess_waits(nc)
    return nc


def host_inputs(x, wq, wk, wv, wo, freqs_cos, freqs_sin):
    """Shard + repack the full inputs into per-core in_maps (bf16)."""
    bf = mybir.dt.np(mybir.dt.bfloat16)
    f32 = np.float32
    cos2 = np.repeat(np.ascontiguousarray(freqs_cos.T), 2, axis=0).astype(bf)
    sin_t = np.ascontiguousarray(freqs_sin.T).astype(f32)
    sin2 = np.empty((P, S), f32)
    sin2[0::2] = -sin_t
    sin2[1::2] = sin_t
    sin2 = sin2.astype(bf)
    fidx = np.arange(QW)
    pidx = np.arange(P)
    cm = np.zeros((P, 4 * QW), f32)
    for j in range(4):
        cm[:, QW * j:QW * (j + 1)] = (
            fidx[None, :] >= (pidx[:, None] + P * j)
        ).astype(f32)
    cm = cm.astype(bf)
    perm = np.zeros((P, P), f32)
    perm[pidx, pidx ^ 1] = 1.0
    perm = perm.astype(bf)
    ident = np.eye(P, dtype=f32).astype(bf)

    in_maps = []
    for c in range(N_CORES):
        b, g = divmod(c, 4)
        xT = np.ascontiguousarray(x[b].T).astype(bf)
        wq_s = wq[512 * g:512 * (g + 1)].astype(f32)      # [512, 2048]
        wq_r = np.ascontiguousarray(
            wq_s.reshape(4, P, KC, P).transpose(3, 0, 2, 1).reshape(P, KC * 4 * P)
        ).astype(bf)
        wk_s = wk[P * g:P * (g + 1)].astype(f32)          # [128, 2048]
        wk_r = np.ascontiguousarray(
            wk_s.reshape(P, KC, P).transpose(2, 1, 0).reshape(P, KC * P)
        ).astype(bf)
        wv_s = wv[P * g:P * (g + 1)].astype(f32)
        wv_r = np.ascontiguousarray(
            wv_s.reshape(P, KC, P).transpose(2, 1, 0).reshape(P, KC * P)
        ).astype(bf)
        woT = np.ascontiguousarray(
            wo[:, 512 * g:512 * (g + 1)].T.astype(f32)).astype(bf)
        in_maps.append({
            "xT": xT, "wq": wq_r, "wk": wk_r, "wv": wv_r, "woT": woT,
            "cos2": cos2, "sin2": sin2, "cm": cm, "perm": perm, "ident": ident,
            "onescol": np.ones((P, 1), f32).astype(bf),
            "onesrow": np.ones((1, P), f32),
            "wup": np.zeros((P, QW), f32).astype(bf),
        })
    return in_maps


def combine_outputs(results):
    out = np.empty((2, S, D), np.float32)
    for b in range(2):
        acc = results[4 * b]["yT"].astype(np.float32)
        for g in range(1, 4):
            acc += results[4 * b + g]["yT"].astype(np.float32)
        out[b] = acc.T
    return out


_NC_CACHE = []


def kernel(x, wq, wk, wv, wo, freqs_cos, freqs_sin, mask):
    del mask  # causal structure handled on-device
    if not _NC_CACHE:
        _NC_CACHE.append(build())
    nc = _NC_CACHE[0]
    in_maps = host_inputs(x, wq, wk, wv, wo, freqs_cos, freqs_sin)
    res = run_bass_kernel_spmd(nc, in_maps, list(range(N_CORES)))
    return combine_outputs(res.results)


# revision 39
# speedup vs baseline: 1.1990x; 1.1990x over previous
"""GQA attention (llama-style, RoPE, causal) on 8 Trainium2 NeuronCores.

Problem: B=2, S=2048, DIM=2048, 16 q-heads / 4 kv-heads, head_dim=128.

Sharding: batch x kv-group. Core c handles batch b=c//4 and kv-group
g=c%4 (q-heads 4g..4g+3, kv-head g). Each core computes its 4 heads'
attention and a partial output projection against wo[:, 512g:512(g+1)];
the host sums the 4 partials per batch. No cross-core communication.

Device-side layout is fully "transposed": activations live as [dim, seq]
so every matmul's contraction dim sits on the SBUF partition axis.
All matmuls run in bf16 (1 cyc/row on PE, same as fp32r, but half the
DMA bytes and 2-4x DVE throughput on the element-wise work).

v2 changes vs the fp32r baseline (339us measured):
 - The per-k-block softmax-sum matmuls (ones^T @ exp, 160 of them, ~36us
   of PE) are gone. Exp tiles accumulate element-wise on the Vector
   engine into two bf16 accumulators (even/odd k-blocks, bounded
   rounding depth); one ones-matmul per head (16 total) does the final
   128-partition reduction. 1/sums via DVE reciprocal_approx_fast.
 - With the sums removed, a head's attention inner loop is 454ns/blk of
   PE vs 612ns/blk of ACT (exp) — so attention alone would be
   ACT-paced. The emission therefore software-pipelines: Q-projection
   of head h+1, K/V projections of chunk c+1 and the output projection
   of chunk c-1 are split into per-matmul "fillers" interleaved between
   attention blocks, keeping PE the pacing engine everywhere.
 - y partials stored as bf16 (half the store drain).
"""

import numpy as np
from contextlib import ExitStack

import bass_rust
import concourse.bass as bass
import concourse.mybir as mybir
import concourse.tile as tile
from concourse.bass_utils import run_bass_kernel_spmd

P = 128          # SBUF partitions / head_dim
S = 2048         # sequence length
D = 2048         # model dim
KC = 16          # contraction chunks of 128 over D
SC = 4           # s-chunks of 512
QW = 512         # moving-operand width
NH = 4           # q-heads per core
N_CORES = 8
SCALE = float(1.0 / np.sqrt(np.float32(128.0)))
F32 = mybir.dt.float32
F32R = mybir.dt.float32r
BF16 = mybir.dt.bfloat16
EXP = mybir.ActivationFunctionType.Exp
LN = mybir.ActivationFunctionType.Ln


class _TC(tile.TileContext):
    """TileContext whose tail drain splits its semaphore waits into
    separate wait instructions — the walrus build here rejects a Drain
    carrying more than a couple of inline sync waits."""

    def _drain_and_barrier(self, tick_clock, wait_clock):
        gc = tick_clock.global_clock
        ticks = [gc[i] for i in range(27)]
        for proc, sem in sorted(self.sems.allocated().items()):
            t = ticks[proc]
            if t > 0:
                mult = 16 if sem.name.startswith(("DMAHW", "DMASW")) else 1
                self.nc.sync.wait_ge(sem, t * mult)
        self.nc.sync.drain()
        self.nc.all_engine_barrier()
        popped = self.nc._tile_sem_poison_stack.pop()
        assert popped is self._sem_poison
        self.nc.clear_and_free_semaphores(list(self.sems.allocated().values()))
        self.nc.all_engine_barrier()


def _split_excess_waits(nc, max_waits=1):
    """This walrus build allows very few inline sync waits per TPB
    instruction. Move excess waits onto injected same-engine NOPs placed
    just before the instruction — semantically identical, since the
    engine queue executes in order."""
    for f in nc.m.functions:
        for blk in f.blocks:
            insts = blk.instructions
            new_list = []
            for inst in insts:
                si = inst.sync_info
                if si is not None and len(si.on_wait) > max_waits:
                    waits = list(si.on_wait)
                    excess, keep = waits[:-max_waits], waits[-max_waits:]
                    for j, w in enumerate(excess):
                        nop = bass_rust.InstNoOp(name=f"{inst.name}-wn{j}")
                        nop.engine = inst.engine
                        nop.sync_info = bass_rust.SyncInfo(
                            on_wait=[w], on_update=[])
                        new_list.append(nop)
                    inst.sync_info = bass_rust.SyncInfo(
                        on_wait=keep, on_update=list(si.on_update))
                new_list.append(inst)
            insts[:] = new_list


def _emit(nc, tc, ctx, t):
    pool = lambda name, bufs, space="SBUF": ctx.enter_context(
        tc.tile_pool(name=name, bufs=bufs, space=space)
    )

    # SBUF pools
    xp = pool("xp", 9)          # x chunk groups [128, 2048] bf16, 2 chunks live
    constp = pool("constp", 1)  # weights, trig tables, masks, resident slabs
    qsbp = pool("qsbp", 2)      # pre-rope proj copy
    t1p = pool("t1p", 2)
    t2p = pool("t2p", 2)
    qrp = pool("qrp", 3)        # rope'd q tiles
    vsbp = pool("vsbp", 1)      # pre-transpose v copy
    ep = pool("ep", 4)          # exp tiles
    eaccp = pool("eaccp", 4)    # exp accumulators (2 per head, 2 heads live)
    rp = pool("rp", 2)          # reciprocal [1, 512]
    rbp = pool("rbp", 2)        # broadcast recip [128, 512]
    otp = pool("otp", 9)        # normalized attention out, 2 chunks live
    yp = pool("yp", 3)          # output copy slabs [128, 1024] bf16

    # PSUM pools — 8 banks total
    pacc = pool("pacc", 2, "PSUM")    # proj accumulators        (2 banks)
    pscore = pool("pscore", 2, "PSUM")  # scoresT                (2 banks)
    pout = pool("pout", 2, "PSUM")    # attention out accum      (2 banks)
    yps = pool("yps", 1, "PSUM")      # output proj y tiles      (1 bank)
    shp = pool("shp", 1, "PSUM")      # rope swap / v transp / bcast / psm (1)

    # resident SBUF slabs
    wq_sb = constp.tile([P, KC * 4 * P], BF16, tag="wq")  # chunk (h,k) at h*2048+k*128
    wk_sb = constp.tile([P, KC * P], BF16, tag="wk")      # chunk k at k*128
    wv_sb = constp.tile([P, KC * P], BF16, tag="wv")
    wo_sb = constp.tile([P, NH * S], BF16, tag="wo")      # chunk (h,dm) at h*2048+dm*128
    cm_sb = constp.tile([P, 4 * QW], BF16, tag="cm")      # 4 causal masks
    cos_sb = constp.tile([P, S], BF16, tag="cos")
    sin_sb = constp.tile([P, S], BF16, tag="sin")
    perm_sb = constp.tile([P, P], BF16, tag="perm")       # pair-swap permutation
    ident_sb = constp.tile([P, P], BF16, tag="ident")
    ones_sb = constp.tile([P, 1], BF16, tag="ones")
    onesrow_sb = constp.tile([1, P], F32R, tag="onesrow")
    wup_sb = constp.tile([P, QW], BF16, tag="wup")        # warmup junk

    kT_sb = constp.tile([P, S], BF16, tag="kT")    # rope'd K^T, filled per s-chunk
    vnat_sb = constp.tile([P, S], BF16, tag="vn")  # V natural [kpos, d], 16 col-blocks

    xT_d, yT_d = t["xT"], t["yT"]

    xgs = {}       # sc -> list of 4 xg tiles (each [128, 4*512])
    qr_tiles = {}  # h -> rope'd q tile for the current chunk
    o_tiles = {}   # (sc, h) -> normalized attention out tile
    norm_ref = {}  # (sc, h) -> (po psum, r recip tile)

    def load_xgroups(sc, split=False):
        tiles = []
        for g in range(4):
            xg = xp.tile([P, 4 * QW], BF16, tag="xg")
            ssl = slice(QW * sc, QW * (sc + 1))
            if split:
                for j in range(4):
                    nc.sync.dma_start(
                        xg[:, QW * j:QW * (j + 1)],
                        xT_d[P * (4 * g + j):P * (4 * g + j + 1), ssl])
            else:
                src = xT_d[4 * P * g:4 * P * (g + 1), ssl].rearrange(
                    "(k p) s -> p k s", p=P)
                nc.sync.dma_start(xg[:].rearrange("p (k s) -> p k s", k=4), src)
            tiles.append(xg)
        xgs[sc] = tiles

    def xs_of(sc):
        g = xgs[sc]
        return [g[k // 4][:, QW * (k % 4):QW * (k % 4 + 1)] for k in range(KC)]

    # ---- filler generators: lists of zero-arg closures, one PE matmul each
    def mk_proj(sc, pi):
        """pi 0..3 = Q head pi (chunk sc); 4 = K (chunk sc); 5 = V (chunk sc).
        Returns 16 mm closures; the 17th emits the post-group chain
        (psum eviction + rope or v-transpose)."""
        st = {}
        xs = None

        def w_ap(k):
            if pi < 4:
                base = pi * 2048 + k * P
                return wq_sb[:, base:base + P]
            if pi == 4:
                return wk_sb[:, k * P:(k + 1) * P]
            return wv_sb[:, k * P:(k + 1) * P]

        def mm(k):
            nonlocal xs
            if k == 0:
                st["ps"] = pacc.tile([P, QW], F32, tag="acc", name="acc")
                xs = xs_of(sc)
            nc.tensor.matmul(st["ps"][:], w_ap(k), xs[k],
                             start=(k == 0), stop=(k == KC - 1))

        def chain():
            ps = st["ps"]
            ssl = slice(QW * sc, QW * (sc + 1))
            if pi == 5:
                # V: psum -> sbuf, then PE-transpose 128-blocks into vnat
                vsb = vsbp.tile([P, QW], BF16, tag="vsb")
                nc.scalar.copy(vsb[:], ps[:])
                for tb in range(4):
                    pt = shp.tile([P, P], BF16, tag="sh")
                    nc.tensor.transpose(pt[:], vsb[:, P * tb:P * (tb + 1)],
                                        ident_sb[:])
                    blk = 4 * sc + tb
                    nc.scalar.copy(vnat_sb[:, P * blk:P * (blk + 1)], pt[:])
            else:
                # Q/K: rope = psum*cos2 + (perm @ psum)*sin2
                qsb = qsbp.tile([P, QW], BF16, tag="qsb")
                nc.scalar.copy(qsb[:], ps[:])
                sw = shp.tile([P, QW], F32, tag="sh")
                nc.tensor.matmul(sw[:], perm_sb[:], qsb[:],
                                 start=True, stop=True)
                t1 = t1p.tile([P, QW], BF16, tag="t1")
                nc.vector.tensor_mul(t1[:], qsb[:], cos_sb[:, ssl])
                t2 = t2p.tile([P, QW], BF16, tag="t2")
                nc.vector.tensor_mul(t2[:], sw[:], sin_sb[:, ssl])
                if pi < 4:
                    dst = qrp.tile([P, QW], BF16, tag="qr")
                    qr_tiles[pi] = dst
                    nc.vector.tensor_add(dst[:], t1[:], t2[:])
                else:
                    nc.vector.tensor_add(kT_sb[:, ssl], t1[:], t2[:])

        return [lambda k=k: mm(k) for k in range(KC)] + [chain]

    def mk_oproj(sc, alt_pool=None):
        """Output projection for chunk sc (64 mm closures; evictions and
        stores ride along on the closing matmul of each 4-mm group).
        alt_pool: alternate even dm groups into another (idle) PSUM pool
        so back-to-back groups don't serialize on the single y bank."""
        st = {}
        ssl = slice(QW * sc, QW * (sc + 1))

        def mm(dm, h):
            if h == 0:
                if alt_pool is not None and dm % 2 == 0:
                    st["py"] = alt_pool.tile([P, QW], F32, tag="score",
                                             name="py")
                else:
                    st["py"] = yps.tile([P, QW], F32, tag="y", name="py")
            nc.tensor.matmul(
                st["py"][:], wo_sb[:, S * h + P * dm:S * h + P * (dm + 1)],
                o_tiles[(sc, h)][:], start=(h == 0), stop=(h == NH - 1))
            if h == NH - 1:
                if dm % 2 == 0:
                    st["ysb"] = yp.tile([P, 2 * QW], BF16, tag="ysb", name="ysb")
                nc.vector.tensor_copy(
                    st["ysb"][:, QW * (dm % 2):QW * (dm % 2 + 1)], st["py"][:])
                if dm % 2 == 1:
                    for j in range(2):
                        dmj = dm - 1 + j
                        nc.sync.dma_start(
                            yT_d[P * dmj:P * (dmj + 1), ssl],
                            st["ysb"][:, QW * j:QW * (j + 1)])

        return [lambda dm=dm, h=h: mm(dm, h) for dm in range(KC) for h in range(NH)]

    def emit_norm(key):
        # broadcast 1/sums across partitions via a K=1 matmul; by the
        # time this runs on PE, r has long been ready (no PE stall)
        po, r = norm_ref.pop(key)
        rbp_ps = shp.tile([P, QW], F32, tag="sh")
        nc.tensor.matmul(rbp_ps[:], onesrow_sb[:], r[:], start=True, stop=True)
        rb = rbp.tile([P, QW], F32, tag="rb")
        nc.vector.tensor_copy(rb[:], rbp_ps[:])
        ot = otp.tile([P, QW], BF16, tag="ot")
        o_tiles[key] = ot
        nc.vector.tensor_mul(ot[:], po[:], rb[:])

    def emit_slot(sc, h, fillers, norm_key):
        """Attention for head h of chunk sc, with fillers interleaved."""
        nkb = 4 * sc + 4
        qr = qr_tiles[h]
        po = pout.tile([P, QW], F32, tag="out")
        eA = eaccp.tile([P, QW], BF16, tag="eacc")
        eB = eaccp.tile([P, QW], BF16, tag="eacc")
        prev = None
        fi = 0

        def pop(n):
            nonlocal fi
            for _ in range(n):
                if fi < len(fillers):
                    fillers[fi]()
                    fi += 1

        def pop_paced(kb):
            # spread remaining fillers over remaining blocks (cap 4/blk),
            # reserving 4 to cover the slot-end exp-sum merge latency
            rem_kb = nkb - kb
            n = min(4, max(0, -(-(len(fillers) - fi - 4) // rem_kb)))
            pop(n)

        # q-columns < 128*kb - 512*sc are causally dead for k-block kb, so
        # the last three k-blocks shrink their moving width (384/256/128)
        qlo = lambda kb: max(0, P * kb - QW * sc)
        for kb in range(nkb):
            ql = qlo(kb)
            qsl = slice(ql, QW)
            psc = pscore.tile([P, QW], F32, tag="score")
            nc.tensor.matmul(psc[:, qsl], kT_sb[:, P * kb:P * (kb + 1)],
                             qr[:, qsl], start=True, stop=True)
            tgt = eA if kb == 0 else eB if kb == 1 else ep.tile(
                [P, QW], BF16, tag="exp")
            nc.scalar.activation(tgt[:, qsl], psc[:, qsl], EXP, scale=SCALE)
            off = P * kb - QW * sc
            if off >= 0:  # diagonal-overlap block: zero kpos > q
                j = off // P
                nc.vector.tensor_mul(tgt[:, qsl], tgt[:, qsl],
                                     cm_sb[:, QW * j + ql:QW * (j + 1)])
            if kb >= 2:
                acc = eA if kb % 2 == 0 else eB
                nc.vector.tensor_add(acc[:, qsl], acc[:, qsl], tgt[:, qsl])
            if kb == min(3, nkb - 1) and norm_key is not None:
                emit_norm(norm_key)
            pop_paced(kb)
            if prev is not None:
                pkb, ptgt = prev
                psl = slice(qlo(pkb), QW)
                nc.tensor.matmul(po[:, psl], vnat_sb[:, P * pkb:P * (pkb + 1)],
                                 ptgt[:, psl], start=(pkb == 0), stop=False)
            prev = (kb, tgt)
        pkb, ptgt = prev
        psl = slice(qlo(pkb), QW)
        nc.tensor.matmul(po[:, psl], vnat_sb[:, P * pkb:P * (pkb + 1)],
                         ptgt[:, psl], start=(pkb == 0), stop=True)
        # final exp-sum: eA += eB on DVE (over eB's covered columns), then
        # one 128->1 reduction matmul. Fillers sit between the DVE merge
        # and the reduction so PE never waits on the merge latency.
        q1 = qlo(1)
        nc.vector.tensor_add(eA[:, q1:], eA[:, q1:], eB[:, q1:])
        pop(4)
        psm = shp.tile([1, QW], F32, tag="sh")
        nc.tensor.matmul(psm[:], ones_sb[:], eA[:], start=True, stop=True)
        # 1/sums = exp(-ln(sums)) on ScalarE (ACT-only: PE keeps going)
        lnr = rp.tile([1, QW], F32, tag="r")
        nc.scalar.activation(lnr[:], psm[:], LN)
        r = rp.tile([1, QW], F32R, tag="r")
        nc.scalar.activation(r[:], lnr[:], EXP, scale=-1.0)
        norm_ref[(sc, h)] = (po, r)
        pop(len(fillers))  # flush

    # ---------------- kernel body ----------------
    # chunk 0 const/weight DMAs; x for chunk 0 split per 128-col chunk so
    # the first K-proj matmuls can start as soon as possible.
    nc.sync.dma_start(perm_sb[:], t["perm"][:])
    nc.sync.dma_start(ident_sb[:], t["ident"][:])
    nc.sync.dma_start(wup_sb[:], t["wup"][:])
    nc.sync.dma_start(ones_sb[:], t["onescol"][:])
    nc.sync.dma_start(onesrow_sb[:], t["onesrow"][:])
    # wk quarters interleaved with the x pieces they pair with
    xg0 = []
    for g in range(4):
        wqt = slice(4 * P * g, 4 * P * (g + 1))
        nc.sync.dma_start(wk_sb[:, wqt], t["wk"][:, wqt])
        xg = xp.tile([P, 4 * QW], BF16, tag="xg", name="xg")
        for j in range(4):
            nc.sync.dma_start(
                xg[:, QW * j:QW * (j + 1)],
                xT_d[P * (4 * g + j):P * (4 * g + j + 1), 0:QW])
        xg0.append(xg)
    xgs[0] = xg0
    for g in range(4):
        wqt = slice(4 * P * g, 4 * P * (g + 1))
        nc.sync.dma_start(wv_sb[:, wqt], t["wv"][:, wqt])
    nc.sync.dma_start(wq_sb[:, 0:2048], t["wq"][:, 0:2048])
    for h in range(1, NH):
        nc.sync.dma_start(wq_sb[:, 2048 * h:2048 * (h + 1)],
                          t["wq"][:, 2048 * h:2048 * (h + 1)])
    nc.sync.dma_start(cos_sb[:], t["cos2"][:])
    nc.sync.dma_start(sin_sb[:], t["sin2"][:])
    nc.sync.dma_start(cm_sb[:], t["cm"][:])
    for h in range(NH):
        nc.sync.dma_start(wo_sb[:, S * h:S * (h + 1)],
                          t["woT"][P * h:P * (h + 1), :])

    # warmup burst: junk matmuls heat the PE clock while DMA ramps
    wupp = shp.tile([P, QW], F32, tag="sh")
    for _ in range(40):
        nc.tensor.matmul(wupp[:], perm_sb[:], wup_sb[:], start=True, stop=True)

    # chunk 0 preamble: K(0), V(0), Q(0,0) contiguous
    for fl in mk_proj(0, 4) + mk_proj(0, 5) + mk_proj(0, 0):
        fl()

    for sc in range(SC):
        if sc < SC - 1:
            load_xgroups(sc + 1)  # prefetch next chunk's x
        # per-slot filler lists. Q(sc,h+1) leads slot h (qr needed next
        # slot); K(sc+1)/Q(sc+1,0) go in slot 2 so their rope chains land
        # well before chunk sc+1's first scores; oproj(sc-1) fills the
        # gaps (its first group is ordered after Q so it pops after the
        # (sc-1,3) normalization has been emitted at kb==3).
        # oproj(sc-1) matmuls are INTERLEAVED with the projection matmuls
        # (not appended) so each 4-matmul y-group spans >=2 attention
        # blocks and its eviction never blocks the next group on the
        # single y PSUM bank. Slot 0's first 12 items stay
        # projection-only so no oproj closure runs before the (sc-1,3)
        # normalization is emitted at kb==3.
        def ilv(a, b):
            out, ia, ib = [], 0, 0
            while ia < len(a) or ib < len(b):
                if ia < len(a):
                    out.append(a[ia]); ia += 1
                if ib < len(b):
                    out.append(b[ib]); ib += 1
            return out

        pr = [mk_proj(sc, 1), mk_proj(sc, 2), mk_proj(sc, 3), []]
        if sc < SC - 1:
            pr[2] += mk_proj(sc + 1, 4) + mk_proj(sc + 1, 0)
            pr[3] += mk_proj(sc + 1, 5)
        if sc > 0:
            op = mk_oproj(sc - 1)
            slot_fill = [
                pr[0][:12] + ilv(pr[0][12:], op[:12]),
                ilv(pr[1], op[12:28]),
                ilv(pr[2], op[28:44]),
                ilv(pr[3], op[44:]),
            ]
        else:
            slot_fill = pr
        for h in range(NH):
            if h > 0:
                norm_key = (sc, h - 1)
            elif sc > 0:
                norm_key = (sc - 1, 3)
            else:
                norm_key = None
            emit_slot(sc, h, slot_fill[h], norm_key)

    # tail: last head's normalization + output projection of chunk 3.
    # pacc/pscore are idle here — run a 4-deep software pipeline across
    # four banks: each group's bank has a full 4-matmul group of slack
    # before reuse, so the eviction latency never blocks PE. Holding each
    # group's closing (h=3) matmul back also hides the (3,3)
    # normalization latency.
    emit_norm((SC - 1, 3))
    ssl3 = slice(QW * (SC - 1), QW * SC)
    pools3 = [pscore, pscore, yps, pacc]
    open_groups = []
    ysb_st = {}

    def close_group(dm, py):
        nc.tensor.matmul(
            py[:], wo_sb[:, S * 3 + P * dm:S * 3 + P * (dm + 1)],
            o_tiles[(SC - 1, 3)][:], start=False, stop=True)
        if dm % 2 == 0:
            ysb_st["ysb"] = yp.tile([P, 2 * QW], BF16, tag="ysb", name="ysb")
        # alternate eviction engine so consecutive groups' copies overlap
        if dm % 2 == 0:
            nc.vector.tensor_copy(
                ysb_st["ysb"][:, QW * (dm % 2):QW * (dm % 2 + 1)], py[:])
        else:
            nc.scalar.copy(
                ysb_st["ysb"][:, QW * (dm % 2):QW * (dm % 2 + 1)], py[:])
        if dm % 2 == 1:
            for j in range(2):
                dmj = dm - 1 + j
                nc.sync.dma_start(yT_d[P * dmj:P * (dmj + 1), ssl3],
                                  ysb_st["ysb"][:, QW * j:QW * (j + 1)])

    tags3 = {id(pscore): "score", id(yps): "y", id(pacc): "acc"}
    for dm in range(KC):
        pl = pools3[dm % 4]
        py = pl.tile([P, QW], F32, tag=tags3[id(pl)], name="py")
        for h in range(3):
            nc.tensor.matmul(
                py[:], wo_sb[:, S * h + P * dm:S * h + P * (dm + 1)],
                o_tiles[(SC - 1, h)][:], start=(h == 0), stop=False)
        open_groups.append((dm, py))
        if len(open_groups) == 4:
            close_group(*open_groups.pop(0))
    while open_groups:
        close_group(*open_groups.pop(0))


def build():
    nc = bass.Bass("TRN2", target_bir_lowering=False, debug=False,
                   num_devices=N_CORES)
    t = {
        "xT": nc.dram_tensor("xT", [D, S], BF16, kind="ExternalInput"),
        "wq": nc.dram_tensor("wq", [P, KC * 4 * P], BF16, kind="ExternalInput"),
        "wk": nc.dram_tensor("wk", [P, KC * P], BF16, kind="ExternalInput"),
        "wv": nc.dram_tensor("wv", [P, KC * P], BF16, kind="ExternalInput"),
        "woT": nc.dram_tensor("woT", [NH * P, S], BF16, kind="ExternalInput"),
        "cos2": nc.dram_tensor("cos2", [P, S], BF16, kind="ExternalInput"),
        "sin2": nc.dram_tensor("sin2", [P, S], BF16, kind="ExternalInput"),
        "cm": nc.dram_tensor("cm", [P, 4 * QW], BF16, kind="ExternalInput"),
        "perm": nc.dram_tensor("perm", [P, P], BF16, kind="ExternalInput"),
        "ident": nc.dram_tensor("ident", [P, P], BF16, kind="ExternalInput"),
        "onescol": nc.dram_tensor("onescol", [P, 1], BF16, kind="ExternalInput"),
        "onesrow": nc.dram_tensor("onesrow", [1, P], F32R, kind="ExternalInput"),
        "wup": nc.dram_tensor("wup", [P, QW], BF16, kind="ExternalInput"),
        "yT": nc.dram_tensor("yT", [D, S], BF16, kind="ExternalOutput"),
    }
    aps = {k: v.ap() for k, v in t.items()}
    with _TC(nc, num_cores=N_CORES) as tc:
        with ExitStack() as ctx:
            _emit(nc, tc, ctx, aps)
    _split_excess_waits(nc)
    return nc


def host_inputs(x, wq, wk, wv, wo, freqs_cos, freqs_sin):
    """Shard + repack the full inputs into per-core in_maps (bf16)."""
    bf = mybir.dt.np(mybir.dt.bfloat16)
    f32 = np.float32
    cos2 = np.repeat(np.ascontiguousarray(freqs_cos.T), 2, axis=0).astype(bf)
    sin_t = np.ascontiguousarray(freqs_sin.T).astype(f32)
    sin2 = np.empty((P, S), f32)
    sin2[0::2] = -sin_t
    sin2[1::2] = sin_t
    sin2 = sin2.astype(bf)
    fidx = np.arange(QW)
    pidx = np.arange(P)
    cm = np.zeros((P, 4 * QW), f32)
    for j in range(4):
        cm[:, QW * j:QW * (j + 1)] = (
            fidx[None, :] >= (pidx[:, None] + P * j)
        ).astype(f32)
    cm = cm.astype(bf)
    perm = np.zeros((P, P), f32)
    perm[pidx, pidx ^ 1] = 1.0
    perm = perm.astype(bf)
    ident = np.eye(P, dtype=f32).astype(bf)

    in_maps = []
    for c in range(N_CORES):
        b, g = divmod(c, 4)
        xT = np.ascontiguousarray(x[b].T).astype(bf)
        wq_s = wq[512 * g:512 * (g + 1)].astype(f32)      # [512, 2048]
        wq_r = np.ascontiguousarray(
            wq_s.reshape(4, P, KC, P).transpose(3, 0, 2, 1).reshape(P, KC * 4 * P)
        ).astype(bf)
        wk_s = wk[P * g:P * (g + 1)].astype(f32)          # [128, 2048]
        wk_r = np.ascontiguousarray(
            wk_s.reshape(P, KC, P).transpose(2, 1, 0).reshape(P, KC * P)
        ).astype(bf)
        wv_s = wv[P * g:P * (g + 1)].astype(f32)
        wv_r = np.ascontiguousarray(
            wv_s.reshape(P, KC, P).transpose(2, 1, 0).reshape(P, KC * P)
        ).astype(bf)
        woT = np.ascontiguousarray(
            wo[:, 512 * g:512 * (g + 1)].T.astype(f32)).astype(bf)
        in_maps.append({
            "xT": xT, "wq": wq_r, "wk": wk_r, "wv": wv_r, "woT": woT,
            "cos2": cos2, "sin2": sin2, "cm": cm, "perm": perm, "ident": ident,
            "onescol": np.ones((P, 1), f32).astype(bf),
            "onesrow": np.ones((1, P), f32),
            "wup": np.zeros((P, QW), f32).astype(bf),
        })
    return in_maps


def combine_outputs(results):
    out = np.empty((2, S, D), np.float32)
    for b in range(2):
        acc = results[4 * b]["yT"].astype(np.float32)
        for g in range(1, 4):
            acc += results[4 * b + g]["yT"].astype(np.float32)
        out[b] = acc.T
    return out


_NC_CACHE = []


def kernel(x, wq, wk, wv, wo, freqs_cos, freqs_sin, mask):
    del mask  # causal structure handled on-device
    if not _NC_CACHE:
        _NC_CACHE.append(build())
    nc = _NC_CACHE[0]
    in_maps = host_inputs(x, wq, wk, wv, wo, freqs_cos, freqs_sin)
    res = run_bass_kernel_spmd(nc, in_maps, list(range(N_CORES)))
    return combine_outputs(res.results)
